# revision 4
# baseline (speedup 1.0000x reference)
"""Contrastive loss (SimCLR-style) on 8 TRN2 NeuronCores.

loss = -mean(diag(log_softmax(zi_n @ zj_n^T / T)))  with zi_n, zj_n L2-normalized,
N=4096, D=256, T=0.5.

v2 design (data-parallel over rows of z_i, z_j replicated):
  - host casts all inputs to bf16 (halves DMA, enables 2x/4x DVE modes).
  - z_i stays unnormalized; its row norm folds into the exp's per-partition
    scale (s2 = 2/|zi|).  zi row norms via ScalarE Square+accum during the
    otherwise-idle prologue.
  - z_j processed in 4 subgroups of 1024 rows: DVE square+accum norms,
    DVE rsqrt bit-trick, row scaling (sg0 on DVE for prologue latency,
    sg1-3 on GpSimd), bf16 xbar DMA transpose.
  - logits: 2 mega-groups x 4 row chunks; PSUM tiles [128, 2048] (4 banks,
    2 bufs); 8 matmuls per tile; exp+row-sum as ONE ScalarE activation per
    2048-wide tile (halves ACT instruction overhead); some tiles offloaded
    to DVE via Schraudolph bit-trick exp (2 passes + accum).
  - PE warmed with ~20 dummy matmuls during the prologue so the HAM clock
    gate is released (2.4 GHz) when the real matmuls start.
  - lse's ln via DVE bit-trick + one exp-based Newton refinement (avoids a
    second ACT table-set load).
  - diagonal via z_jd input: unnormalized dot + two per-partition fixups.
  - each core returns 4 partial sums of (lse[n] - diag[n]); host adds the
    32 values and divides by N.
"""

import numpy as np
import ml_dtypes

import concourse.bass as bass
import concourse.bacc as bacc
import concourse.tile as tile
import concourse.bass_utils as bass_utils
from concourse import mybir

N = 4096
D = 256
NCORES = 8
NL = N // NCORES  # 512 local rows per core
P = 128
NCHUNK = NL // P  # 4 local row chunks
SG = 4  # zj subgroups
SGM = N // SG  # 1024 rows per subgroup
GCH = SGM // P  # 8 chunks per subgroup
KH = D // P  # 2 contraction halves
MW = 2048  # psum tile width (one mega-group of 2048 zj rows)
MAGIC = 0x5F3759DF

# Schraudolph exp / log constants
EXP_A2 = 2.0 * 8388608.0 / 0.6931471805599453  # 2 * 2^23/ln2 (temp fold)
EXP_B = 1064872509.0  # 127*2^23 - 480707 (mean-balanced)
LN_B = 1064872509.0
LN_S = 0.6931471805599453 / 8388608.0  # ln2 / 2^23

# (mg, i) logits tiles whose exp runs on DVE instead of ScalarE
DVE_TILES = {(1, 3)}
WARM_MMS = 20

F32 = mybir.dt.float32
I32 = mybir.dt.int32
U32 = mybir.dt.uint32
BF16 = mybir.dt.bfloat16
AF = mybir.ActivationFunctionType
ALU = mybir.AluOpType


def build_nc():
    nc = bacc.Bacc(
        "TRN2",
        target_bir_lowering=False,
        debug=False,
        enable_asserts=False,
    )
    z_i = nc.dram_tensor("z_i", (NL, D), BF16, kind="ExternalInput").ap()
    z_j = nc.dram_tensor("z_j", (N, D), BF16, kind="ExternalInput").ap()
    z_jd = nc.dram_tensor("z_jd", (NL, D), BF16, kind="ExternalInput").ap()
    out = nc.dram_tensor("out", (1, NCHUNK), F32, kind="ExternalOutput").ap()

    with tile.TileContext(nc) as tc:
        with (
            tc.tile_pool(name="const", bufs=1) as const,
            tc.tile_pool(name="big", bufs=1) as big,
            tc.tile_pool(name="work", bufs=2) as work,
            tc.tile_pool(name="stat", bufs=1) as stat,
            tc.tile_pool(name="psum", bufs=2, space="PSUM") as psum,
        ):
            # --- constants + ACT exp table load at t=0
            dummy = const.tile([1, 1], F32)
            nc.vector.memset(dummy, 1.0)
            nc.scalar.activation(out=dummy, in_=dummy, func=AF.Exp)

            ones = const.tile([P, 1], F32)
            nc.vector.memset(ones, 1.0)
            magic = const.tile([P, GCH], U32)
            nc.vector.memset(magic, MAGIC)
            warm_rhs = const.tile([P, 512], BF16)
            nc.vector.memset(warm_rhs, 0.0)
            ones_bf = const.tile([P, 1], BF16)
            nc.vector.memset(ones_bf, 1.0)

            # --- PE warmup: release the HAM clock gate before real matmuls
            warm_pt = psum.tile([P, MW], F32, tag="pt")
            for _ in range(WARM_MMS):
                nc.tensor.matmul(
                    warm_pt[:1, :512], lhsT=ones_bf, rhs=warm_rhs,
                    start=True, stop=True,
                )

            def rsqrt_dve(a, y, w):
                """y[:,:w] = 1/sqrt(a[:,:w]): quake seed + 1 Newton step."""
                au = a.bitcast(U32)
                yu = y.bitcast(U32)
                sh = work.tile([P, GCH], U32, tag="rsq_sh")
                nc.vector.tensor_scalar(
                    out=sh[:, :w], in0=au, scalar1=1, scalar2=None,
                    op0=ALU.logical_shift_right,
                )
                nc.vector.tensor_sub(out=yu, in0=magic[:, :w], in1=sh[:, :w])
                t1 = work.tile([P, GCH], F32, tag="rsq_t1")
                nc.vector.tensor_mul(out=t1[:, :w], in0=y, in1=y)
                nc.vector.tensor_mul(out=t1[:, :w], in0=t1[:, :w], in1=a)
                nc.vector.tensor_scalar(
                    out=t1[:, :w], in0=t1[:, :w], scalar1=-0.5, scalar2=1.5,
                    op0=ALU.mult, op1=ALU.add,
                )
                nc.vector.tensor_mul(out=y, in0=y, in1=t1[:, :w])

            # --- zi: bf16 load; row norms on ScalarE (prologue is ACT-idle)
            zi_bf = big.tile([P, NCHUNK, D], BF16)
            nc.scalar.dma_start(
                out=zi_bf, in_=z_i.rearrange("(c p) d -> p c d", p=P)
            )
            ziT = big.tile([P, NCHUNK * KH, P], BF16)
            nc.sync.dma_start_transpose(
                out=ziT, in_=zi_bf.rearrange("p c d -> p (c d)")
            )
            ziT_r = ziT.rearrange("do (i h) m -> do i h m", h=KH)

            nrm2_i = stat.tile([P, NCHUNK], F32)
            junk_act = work.tile([P, D], BF16, tag="junk_act")
            for i in range(NCHUNK):
                nc.scalar.activation(
                    out=junk_act, in_=zi_bf[:, i, :], func=AF.Square,
                    accum_out=nrm2_i[:, i : i + 1],
                )
            t_i = stat.tile([P, NCHUNK], F32)
            rsqrt_dve(nrm2_i, t_i, NCHUNK)
            s2 = stat.tile([P, NCHUNK], F32)
            nc.vector.tensor_scalar(
                out=s2, in0=t_i, scalar1=2.0, scalar2=None, op0=ALU.mult
            )
            As2 = stat.tile([P, NCHUNK], F32)
            nc.vector.tensor_scalar(
                out=As2, in0=t_i, scalar1=EXP_A2, scalar2=None, op0=ALU.mult
            )

            # --- per-subgroup zj prep
            nrm2_j = stat.tile([P, SG * GCH], F32)
            t_j = stat.tile([P, SG * GCH], F32)
            zjT_r = []

            def zj_group(g):
                zj_f = big.tile([P, GCH, D], BF16, tag=f"zjf{g}")
                nc.scalar.dma_start(
                    out=zj_f,
                    in_=z_j[g * SGM : (g + 1) * SGM, :].rearrange(
                        "(c p) d -> p c d", p=P
                    ),
                )
                for jl in range(GCH):
                    j = g * GCH + jl
                    sq = work.tile([P, D], BF16, tag="sq")
                    nc.vector.scalar_tensor_tensor(
                        out=sq, in0=zj_f[:, jl, :], scalar=1.0,
                        in1=zj_f[:, jl, :],
                        op0=ALU.mult, op1=ALU.mult,
                        accum_out=nrm2_j[:, j : j + 1],
                    )
                gs = slice(g * GCH, (g + 1) * GCH)
                rsqrt_dve(nrm2_j[:, gs], t_j[:, gs], GCH)
                zjs = big.tile([P, GCH, D], BF16, tag=f"zjs{g}")
                eng = nc.vector if g == 0 else nc.gpsimd
                for jl in range(GCH):
                    j = g * GCH + jl
                    eng.tensor_scalar_mul(
                        out=zjs[:, jl, :],
                        in0=zj_f[:, jl, :],
                        scalar1=t_j[:, j : j + 1],
                    )
                zjT = big.tile([P, GCH * KH, P], BF16, tag=f"zjT{g}")
                nc.sync.dma_start_transpose(
                    out=zjT, in_=zjs.rearrange("p c d -> p (c d)")
                )
                zjT_r.append(zjT.rearrange("do (c h) m -> do c h m", h=KH))

            for g in range(SG):
                zj_group(g)

            # --- diagonal block: unnormalized dot, norms fixed up per-partition
            zjd_f = big.tile([P, NCHUNK, D], BF16)
            nc.scalar.dma_start(
                out=zjd_f, in_=z_jd.rearrange("(c p) d -> p c d", p=P)
            )
            nrm2_d = stat.tile([P, NCHUNK], F32)
            for i in range(NCHUNK):
                sq = work.tile([P, D], BF16, tag="sq")
                nc.vector.scalar_tensor_tensor(
                    out=sq, in0=zjd_f[:, i, :], scalar=1.0, in1=zjd_f[:, i, :],
                    op0=ALU.mult, op1=ALU.mult,
                    accum_out=nrm2_d[:, i : i + 1],
                )
            t_d = stat.tile([P, NCHUNK], F32)
            rsqrt_dve(nrm2_d, t_d, NCHUNK)
            dt = stat.tile([P, NCHUNK], F32)
            for i in range(NCHUNK):
                sq = work.tile([P, D], BF16, tag="sq")
                nc.vector.scalar_tensor_tensor(
                    out=sq, in0=zi_bf[:, i, :], scalar=1.0, in1=zjd_f[:, i, :],
                    op0=ALU.mult, op1=ALU.mult,
                    accum_out=dt[:, i : i + 1],
                )
            dg0 = stat.tile([P, NCHUNK], F32)
            nc.vector.tensor_mul(out=dg0, in0=dt, in1=t_d)
            dg = stat.tile([P, NCHUNK], F32)
            nc.vector.tensor_mul(out=dg, in0=dg0, in1=s2)

            # --- main compute: [128, 2048] logits tiles, 8 MMs each, fused exp
            lse_parts = stat.tile([P, 2 * NCHUNK], F32)
            i32t = big.tile([P, MW], I32)
            junk_bf = big.tile([P, MW], BF16)

            def logits_tile(mg, i):
                k = mg * NCHUNK + i
                pt = psum.tile([P, MW], F32, tag="pt")
                for half in range(2):
                    g = mg * 2 + half
                    for h in range(KH):
                        for jj in range(2):
                            c0 = jj * 4
                            off = half * 1024 + jj * 512
                            nc.tensor.matmul(
                                pt[:, off : off + 512],
                                lhsT=ziT_r[:, i, h, :],
                                rhs=zjT_r[g][:, c0 : c0 + 4, h, :],
                                start=(h == 0),
                                stop=(h == KH - 1),
                            )
                if (mg, i) in DVE_TILES:
                    nc.vector.tensor_scalar(
                        out=i32t, in0=pt, scalar1=As2[:, i : i + 1],
                        scalar2=EXP_B, op0=ALU.mult, op1=ALU.add,
                    )
                    nc.vector.tensor_scalar(
                        out=junk_bf, in0=i32t.bitcast(F32), scalar1=1.0,
                        scalar2=0.0, op0=ALU.mult, op1=ALU.add,
                        accum_out=lse_parts[:, k : k + 1],
                    )
                else:
                    nc.scalar.activation(
                        out=pt, in_=pt, func=AF.Exp,
                        scale=s2[:, i : i + 1],
                        accum_out=lse_parts[:, k : k + 1],
                    )

            for mg in range(2):
                for i in range(NCHUNK):
                    logits_tile(mg, i)

            # --- lse = ln(S) via bit-trick + one Newton refinement
            rs = stat.tile([P, NCHUNK], F32)
            nc.vector.tensor_add(
                out=rs, in0=lse_parts[:, :NCHUNK], in1=lse_parts[:, NCHUNK:]
            )
            vf = stat.tile([P, NCHUNK], F32)
            nc.vector.tensor_copy(out=vf, in_=rs.bitcast(I32))
            lse0 = stat.tile([P, NCHUNK], F32)
            nc.vector.tensor_scalar(
                out=lse0, in0=vf, scalar1=LN_B, scalar2=LN_S,
                op0=ALU.subtract, op1=ALU.mult,
            )
            ef = stat.tile([P, NCHUNK], F32)
            nc.scalar.activation(out=ef, in_=lse0, func=AF.Exp, scale=-1.0)
            rf = stat.tile([P, NCHUNK], F32)
            nc.vector.tensor_mul(out=rf, in0=rs, in1=ef)
            uf = stat.tile([P, NCHUNK], F32)
            nc.vector.tensor_scalar(
                out=uf, in0=rf, scalar1=1.0, scalar2=None, op0=ALU.subtract
            )
            qf = stat.tile([P, NCHUNK], F32)
            nc.vector.scalar_tensor_tensor(
                out=qf, in0=uf, scalar=-0.5, in1=uf,
                op0=ALU.mult, op1=ALU.mult,
            )
            l1 = stat.tile([P, NCHUNK], F32)
            nc.vector.tensor_add(out=l1, in0=lse0, in1=uf)
            lse = stat.tile([P, NCHUNK], F32)
            nc.vector.tensor_add(out=lse, in0=l1, in1=qf)
            contrib = stat.tile([P, NCHUNK], F32)
            nc.vector.tensor_sub(out=contrib, in0=lse, in1=dg)

            # --- partition reduction via ones-matmul: [1, 4] partials
            pt_fin = psum.tile([P, MW], F32, tag="pt")
            nc.tensor.matmul(
                pt_fin[:1, :NCHUNK], lhsT=ones, rhs=contrib, start=True, stop=True
            )
            osb = stat.tile([1, NCHUNK], F32)
            nc.vector.tensor_copy(out=osb, in_=pt_fin[:1, :NCHUNK])
            nc.sync.dma_start(out=out, in_=osb)

    nc.compile()
    return nc


_NC = None


def _get_nc():
    global _NC
    if _NC is None:
        _NC = build_nc()
    return _NC


def make_in_maps(z_i: np.ndarray, z_j: np.ndarray):
    z_i = np.asarray(z_i).astype(ml_dtypes.bfloat16)
    z_j = np.asarray(z_j).astype(ml_dtypes.bfloat16)
    in_maps = []
    for c in range(NCORES):
        sl = slice(c * NL, (c + 1) * NL)
        in_maps.append(
            {
                "z_i": np.ascontiguousarray(z_i[sl]),
                "z_j": z_j,
                "z_jd": np.ascontiguousarray(z_j[sl]),
            }
        )
    return in_maps


def kernel(z_i: np.ndarray, z_j: np.ndarray, **_unused) -> np.ndarray:
    nc = _get_nc()
    in_maps = make_in_maps(z_i, z_j)
    res = bass_utils.run_bass_kernel_spmd(
        nc, in_maps, core_ids=list(range(NCORES))
    )
    total = 0.0
    for c in range(NCORES):
        total += float(res.results[c]["out"].astype(np.float64).sum())
    return np.float32(total / N)


# revision 9
# speedup vs baseline: 2.1731x; 2.1731x over previous
"""Contrastive loss (SimCLR-style) on 8 TRN2 NeuronCores.

loss = -mean(diag(log_softmax(zi_n @ zj_n^T / T)))  with zi_n, zj_n L2-normalized,
N=4096, D=256, T=0.5.

v2 design (data-parallel over rows of z_i, z_j replicated):
  - host casts all inputs to bf16 (halves DMA, enables 2x/4x DVE modes).
  - z_i stays unnormalized; its row norm folds into the exp's per-partition
    scale (s2 = 2/|zi|).  zi row norms via ScalarE Square+accum during the
    otherwise-idle prologue.
  - z_j processed in 4 subgroups of 1024 rows: DVE square+accum norms,
    DVE rsqrt bit-trick, row scaling (sg0 on DVE for prologue latency,
    sg1-3 on GpSimd), bf16 xbar DMA transpose.
  - logits: 2 mega-groups x 4 row chunks; PSUM tiles [128, 2048] (4 banks,
    2 bufs); 8 matmuls per tile; exp+row-sum as ONE ScalarE activation per
    2048-wide tile (halves ACT instruction overhead); some tiles offloaded
    to DVE via Schraudolph bit-trick exp (2 passes + accum).
  - PE warmed with ~20 dummy matmuls during the prologue so the HAM clock
    gate is released (2.4 GHz) when the real matmuls start.
  - lse's ln via DVE bit-trick + one exp-based Newton refinement (avoids a
    second ACT table-set load).
  - diagonal via z_jd input: unnormalized dot + two per-partition fixups.
  - each core returns 4 partial sums of (lse[n] - diag[n]); host adds the
    32 values and divides by N.
"""

import numpy as np
import ml_dtypes

import concourse.bass as bass
import concourse.bacc as bacc
import concourse.tile as tile
import concourse.bass_utils as bass_utils
from concourse import mybir

N = 4096
D = 256
NCORES = 8
NL = N // NCORES  # 512 local rows per core
P = 128
NCHUNK = NL // P  # 4 local row chunks
SG = 4  # zj subgroups
SGM = N // SG  # 1024 rows per subgroup
GCH = SGM // P  # 8 chunks per subgroup
KH = D // P  # 2 contraction halves
MW = 2048  # psum tile width (one mega-group of 2048 zj rows)
MAGIC = 0x5F3759DF

# Schraudolph exp / log constants
EXP_A2 = 2.0 * 8388608.0 / 0.6931471805599453  # 2 * 2^23/ln2 (temp fold)
EXP_B = 1064872509.0  # 127*2^23 - 480707 (mean-balanced)
LN_B = 1064872509.0
LN_S = 0.6931471805599453 / 8388608.0  # ln2 / 2^23

# (mg, i) logits tiles whose exp runs on DVE instead of ScalarE
DVE_TILES = set()
WARM_MMS = 8

F32 = mybir.dt.float32
I32 = mybir.dt.int32
U32 = mybir.dt.uint32
BF16 = mybir.dt.bfloat16
AF = mybir.ActivationFunctionType
ALU = mybir.AluOpType
AX = mybir.AxisListType


def build_nc():
    nc = bacc.Bacc(
        "TRN2",
        target_bir_lowering=False,
        debug=False,
        enable_asserts=False,
    )
    z_i = nc.dram_tensor("z_i", (NL, D), BF16, kind="ExternalInput").ap()
    z_j = nc.dram_tensor("z_j", (N, D), BF16, kind="ExternalInput").ap()
    z_jd = nc.dram_tensor("z_jd", (NL, D), BF16, kind="ExternalInput").ap()
    out = nc.dram_tensor("out", (1, NCHUNK), F32, kind="ExternalOutput").ap()

    with tile.TileContext(nc) as tc:
        with (
            tc.tile_pool(name="const", bufs=1) as const,
            tc.tile_pool(name="big", bufs=1) as big,
            tc.tile_pool(name="work", bufs=2) as work,
            tc.tile_pool(name="stat", bufs=1) as stat,
            tc.tile_pool(name="psum", bufs=2, space="PSUM") as psum,
        ):
            # --- constants + ACT exp table load at t=0
            dummy = const.tile([1, 1], F32)
            nc.vector.memset(dummy, 1.0)
            nc.scalar.activation(out=dummy, in_=dummy, func=AF.Exp)

            ones = const.tile([P, 1], F32)
            nc.vector.memset(ones, 1.0)
            magic = const.tile([P, GCH], U32)
            nc.vector.memset(magic, MAGIC)
            warm_rhs = const.tile([P, 512], BF16)
            nc.vector.memset(warm_rhs, 0.0)
            ones_bf = const.tile([P, 1], BF16)
            nc.vector.memset(ones_bf, 1.0)

            # --- PE warmup: release the HAM clock gate before real matmuls
            warm_pt = psum.tile([P, MW], F32, tag="pt")
            for w in range(WARM_MMS):
                off = (w % 4) * 512
                nc.tensor.matmul(
                    warm_pt[:1, off : off + 512], lhsT=ones_bf, rhs=warm_rhs,
                    start=True, stop=True,
                )

            def rsqrt_dve(a, y, w):
                """y[:,:w] = 1/sqrt(a[:,:w]): quake seed + 1 Newton step."""
                au = a.bitcast(U32)
                yu = y.bitcast(U32)
                sh = work.tile([P, GCH], U32, tag="rsq_sh")
                nc.vector.tensor_scalar(
                    out=sh[:, :w], in0=au, scalar1=1, scalar2=None,
                    op0=ALU.logical_shift_right,
                )
                nc.vector.tensor_sub(out=yu, in0=magic[:, :w], in1=sh[:, :w])
                t1 = work.tile([P, GCH], F32, tag="rsq_t1")
                nc.vector.tensor_mul(out=t1[:, :w], in0=y, in1=y)
                nc.vector.tensor_mul(out=t1[:, :w], in0=t1[:, :w], in1=a)
                nc.vector.tensor_scalar(
                    out=t1[:, :w], in0=t1[:, :w], scalar1=-0.5, scalar2=1.5,
                    op0=ALU.mult, op1=ALU.add,
                )
                nc.vector.tensor_mul(out=y, in0=y, in1=t1[:, :w])

            # --- zi: bf16 load; row norms on ScalarE (prologue is ACT-idle)
            zi_bf = big.tile([P, NCHUNK, D], BF16)
            nc.scalar.dma_start(
                out=zi_bf, in_=z_i.rearrange("(c p) d -> p c d", p=P)
            )
            ziT = big.tile([P, NCHUNK * KH, P], BF16)
            nc.sync.dma_start_transpose(
                out=ziT, in_=zi_bf.rearrange("p c d -> p (c d)")
            )
            ziT_r = ziT.rearrange("do (i h) m -> do i h m", h=KH)

            nrm2_i = stat.tile([P, NCHUNK], F32)
            junk_act = work.tile([P, D], BF16, tag="junk_act")
            for i in range(NCHUNK):
                nc.scalar.activation(
                    out=junk_act, in_=zi_bf[:, i, :], func=AF.Square,
                    accum_out=nrm2_i[:, i : i + 1],
                )
            t_i = stat.tile([P, NCHUNK], F32)
            rsqrt_dve(nrm2_i, t_i, NCHUNK)
            s2 = stat.tile([P, NCHUNK], F32)
            nc.vector.tensor_scalar(
                out=s2, in0=t_i, scalar1=2.0, scalar2=None, op0=ALU.mult
            )
            As2 = stat.tile([P, NCHUNK], F32)
            nc.vector.tensor_scalar(
                out=As2, in0=t_i, scalar1=EXP_A2, scalar2=None, op0=ALU.mult
            )

            # --- per-subgroup zj prep
            nrm2_j = stat.tile([P, SG * GCH], F32)
            t_j = stat.tile([P, SG * GCH], F32)
            zjT_r = []

            def zj_group(g):
                zj_f = big.tile([P, GCH, D], BF16, tag=f"zjf{g}")
                nc.scalar.dma_start(
                    out=zj_f,
                    in_=z_j[g * SGM : (g + 1) * SGM, :].rearrange(
                        "(c p) d -> p c d", p=P
                    ),
                )
                gs = slice(g * GCH, (g + 1) * GCH)
                sqw = work.tile([P, GCH, D], BF16, tag="sqw")
                nc.vector.tensor_mul(out=sqw, in0=zj_f, in1=zj_f)
                nc.vector.tensor_reduce(
                    out=nrm2_j[:, gs], in_=sqw, axis=AX.X, op=ALU.add
                )
                rsqrt_dve(nrm2_j[:, gs], t_j[:, gs], GCH)
                zjs = big.tile([P, GCH, D], BF16, tag=f"zjs{g}")
                for jl in range(GCH):
                    j = g * GCH + jl
                    nc.vector.tensor_scalar_mul(
                        out=zjs[:, jl, :],
                        in0=zj_f[:, jl, :],
                        scalar1=t_j[:, j : j + 1],
                    )
                zjT = big.tile([P, GCH * KH, P], BF16, tag=f"zjT{g}")
                nc.sync.dma_start_transpose(
                    out=zjT, in_=zjs.rearrange("p c d -> p (c d)")
                )
                zjT_r.append(zjT.rearrange("do (c h) m -> do c h m", h=KH))

            for g in range(SG):
                zj_group(g)

            # --- diagonal block: unnormalized dot, norms fixed up per-partition
            zjd_f = big.tile([P, NCHUNK, D], BF16)
            nc.scalar.dma_start(
                out=zjd_f, in_=z_jd.rearrange("(c p) d -> p c d", p=P)
            )
            nrm2_d = stat.tile([P, NCHUNK], F32)
            sqd = work.tile([P, NCHUNK, D], BF16, tag="sqd")
            nc.vector.tensor_mul(out=sqd, in0=zjd_f, in1=zjd_f)
            nc.vector.tensor_reduce(
                out=nrm2_d, in_=sqd, axis=AX.X, op=ALU.add
            )
            t_d = stat.tile([P, NCHUNK], F32)
            rsqrt_dve(nrm2_d, t_d, NCHUNK)
            dt = stat.tile([P, NCHUNK], F32)
            dprod = work.tile([P, NCHUNK, D], BF16, tag="sqd")
            nc.vector.tensor_mul(out=dprod, in0=zi_bf, in1=zjd_f)
            nc.vector.tensor_reduce(
                out=dt, in_=dprod, axis=AX.X, op=ALU.add
            )
            dg0 = stat.tile([P, NCHUNK], F32)
            nc.vector.tensor_mul(out=dg0, in0=dt, in1=t_d)
            dg = stat.tile([P, NCHUNK], F32)
            nc.vector.tensor_mul(out=dg, in0=dg0, in1=s2)

            # --- main compute: [128, 2048] logits tiles, 8 MMs each, fused exp
            lse_parts = stat.tile([P, 2 * NCHUNK], F32)
            i32t = big.tile([P, MW], I32)
            junk_bf = big.tile([P, MW], BF16)

            def logits_tile(mg, i):
                k = mg * NCHUNK + i
                pt = psum.tile([P, MW], F32, tag="pt")
                for half in range(2):
                    g = mg * 2 + half
                    for h in range(KH):
                        for jj in range(2):
                            c0 = jj * 4
                            off = half * 1024 + jj * 512
                            nc.tensor.matmul(
                                pt[:, off : off + 512],
                                lhsT=ziT_r[:, i, h, :],
                                rhs=zjT_r[g][:, c0 : c0 + 4, h, :],
                                start=(h == 0),
                                stop=(h == KH - 1),
                            )
                if (mg, i) in DVE_TILES:
                    nc.vector.tensor_scalar(
                        out=i32t, in0=pt, scalar1=As2[:, i : i + 1],
                        scalar2=EXP_B, op0=ALU.mult, op1=ALU.add,
                    )
                    nc.vector.tensor_scalar(
                        out=junk_bf, in0=i32t.bitcast(F32), scalar1=1.0,
                        scalar2=0.0, op0=ALU.mult, op1=ALU.add,
                        accum_out=lse_parts[:, k : k + 1],
                    )
                else:
                    nc.scalar.activation(
                        out=pt, in_=pt, func=AF.Exp,
                        scale=s2[:, i : i + 1],
                        accum_out=lse_parts[:, k : k + 1],
                    )

            for mg in range(2):
                for i in range(NCHUNK):
                    logits_tile(mg, i)

            # --- lse = ln(S) via bit-trick + one Newton refinement
            rs = stat.tile([P, NCHUNK], F32)
            nc.vector.tensor_add(
                out=rs, in0=lse_parts[:, :NCHUNK], in1=lse_parts[:, NCHUNK:]
            )
            vf = stat.tile([P, NCHUNK], F32)
            nc.vector.tensor_copy(out=vf, in_=rs.bitcast(I32))
            lse0 = stat.tile([P, NCHUNK], F32)
            nc.vector.tensor_scalar(
                out=lse0, in0=vf, scalar1=LN_B, scalar2=LN_S,
                op0=ALU.subtract, op1=ALU.mult,
            )
            ef = stat.tile([P, NCHUNK], F32)
            nc.scalar.activation(out=ef, in_=lse0, func=AF.Exp, scale=-1.0)
            rf = stat.tile([P, NCHUNK], F32)
            nc.vector.tensor_mul(out=rf, in0=rs, in1=ef)
            uf = stat.tile([P, NCHUNK], F32)
            nc.vector.tensor_scalar(
                out=uf, in0=rf, scalar1=1.0, scalar2=None, op0=ALU.subtract
            )
            qf = stat.tile([P, NCHUNK], F32)
            nc.vector.scalar_tensor_tensor(
                out=qf, in0=uf, scalar=-0.5, in1=uf,
                op0=ALU.mult, op1=ALU.mult,
            )
            l1 = stat.tile([P, NCHUNK], F32)
            nc.vector.tensor_add(out=l1, in0=lse0, in1=uf)
            lse = stat.tile([P, NCHUNK], F32)
            nc.vector.tensor_add(out=lse, in0=l1, in1=qf)
            contrib = stat.tile([P, NCHUNK], F32)
            nc.vector.tensor_sub(out=contrib, in0=lse, in1=dg)

            # --- partition reduction via ones-matmul: [1, 4] partials
            pt_fin = psum.tile([P, MW], F32, tag="pt")
            nc.tensor.matmul(
                pt_fin[:1, :NCHUNK], lhsT=ones, rhs=contrib, start=True, stop=True
            )
            osb = stat.tile([1, NCHUNK], F32)
            nc.vector.tensor_copy(out=osb, in_=pt_fin[:1, :NCHUNK])
            nc.sync.dma_start(out=out, in_=osb)

    nc.compile()
    return nc


_NC = None


def _get_nc():
    global _NC
    if _NC is None:
        _NC = build_nc()
    return _NC


def make_in_maps(z_i: np.ndarray, z_j: np.ndarray):
    z_i = np.asarray(z_i).astype(ml_dtypes.bfloat16)
    z_j = np.asarray(z_j).astype(ml_dtypes.bfloat16)
    in_maps = []
    for c in range(NCORES):
        sl = slice(c * NL, (c + 1) * NL)
        in_maps.append(
            {
                "z_i": np.ascontiguousarray(z_i[sl]),
                "z_j": z_j,
                "z_jd": np.ascontiguousarray(z_j[sl]),
            }
        )
    return in_maps


def kernel(z_i: np.ndarray, z_j: np.ndarray, **_unused) -> np.ndarray:
    nc = _get_nc()
    in_maps = make_in_maps(z_i, z_j)
    res = bass_utils.run_bass_kernel_spmd(
        nc, in_maps, core_ids=list(range(NCORES))
    )
    total = 0.0
    for c in range(NCORES):
        total += float(res.results[c]["out"].astype(np.float64).sum())
    return np.float32(total / N)


# revision 10
# speedup vs baseline: 2.2336x; 1.0279x over previous
"""Contrastive loss (SimCLR-style) on 8 TRN2 NeuronCores.

loss = -mean(diag(log_softmax(zi_n @ zj_n^T / T)))  with zi_n, zj_n L2-normalized,
N=4096, D=256, T=0.5.

v4 design (data-parallel over rows of z_i, z_j replicated):
  - host casts all inputs to bf16.
  - z_i unnormalized; row norm folds into the exp's per-partition scale
    (s2 = 2/|zi|).  Norms via one wide DVE square + tensor_reduce.
  - z_j in 4 subgroups of 1024 rows; per sg: wide bf16 square (TT 2x) +
    one tensor_reduce + rsqrt bit-trick + 8 per-chunk scales, then a bf16
    xbar DMA transpose.  All on DVE; chain is the kernel's pacing item, so
    compute starts as soon as sg0 is ready.
  - logits/exp in 3 waves to minimize both start latency and tail:
    sg0 as 4 single [128,1024] tiles, (sg1,sg2) as 4 paired [128,2048]
    tiles, sg3 as 4 single tiles.  exp+row-sum fused on ScalarE with
    per-partition scale; in-place over PSUM.
  - zjd diag block: squares on ScalarE inside its idle bubble; dot and
    fixups (dt * t_d * s2) on DVE.
  - lse's ln via DVE bit-trick + one exp-based Newton refinement (no 2nd
    ACT table-set load).
  - PE warmed with dummy matmuls so HAM is at 2.4 GHz for the real MMs.
  - each core returns 4 partial sums of (lse[n] - diag[n]); host adds the
    32 values and divides by N.
"""

import numpy as np
import ml_dtypes

import concourse.bass as bass
import concourse.bacc as bacc
import concourse.tile as tile
import concourse.bass_utils as bass_utils
from concourse import mybir

N = 4096
D = 256
NCORES = 8
NL = N // NCORES  # 512 local rows per core
P = 128
NCHUNK = NL // P  # 4 local row chunks
SG = 4  # zj subgroups
SGM = N // SG  # 1024 rows per subgroup
GCH = SGM // P  # 8 chunks per subgroup
KH = D // P  # 2 contraction halves
MAGIC = 0x5F3759DF

LN_B = 1064872509.0
LN_S = 0.6931471805599453 / 8388608.0  # ln2 / 2^23

WARM_MMS = 8

F32 = mybir.dt.float32
I32 = mybir.dt.int32
U32 = mybir.dt.uint32
BF16 = mybir.dt.bfloat16
AF = mybir.ActivationFunctionType
ALU = mybir.AluOpType
AX = mybir.AxisListType


def build_nc():
    nc = bacc.Bacc(
        "TRN2",
        target_bir_lowering=False,
        debug=False,
        enable_asserts=False,
    )
    z_i = nc.dram_tensor("z_i", (NL, D), BF16, kind="ExternalInput").ap()
    z_j = nc.dram_tensor("z_j", (N, D), BF16, kind="ExternalInput").ap()
    z_jd = nc.dram_tensor("z_jd", (NL, D), BF16, kind="ExternalInput").ap()
    out = nc.dram_tensor("out", (1, NCHUNK), F32, kind="ExternalOutput").ap()

    with tile.TileContext(nc) as tc:
        with (
            tc.tile_pool(name="const", bufs=1) as const,
            tc.tile_pool(name="big", bufs=1) as big,
            tc.tile_pool(name="work", bufs=2) as work,
            tc.tile_pool(name="stat", bufs=1) as stat,
            tc.tile_pool(name="psum1", bufs=2, space="PSUM") as psum1,
            tc.tile_pool(name="psum2", bufs=1, space="PSUM") as psum2,
        ):
            # --- constants + ACT exp table load at t=0
            dummy = const.tile([1, 1], F32)
            nc.vector.memset(dummy, 1.0)
            nc.scalar.activation(out=dummy, in_=dummy, func=AF.Exp)

            ones = const.tile([P, 1], F32)
            nc.vector.memset(ones, 1.0)
            magic = const.tile([P, GCH], U32)
            nc.vector.memset(magic, MAGIC)
            warm_rhs = const.tile([P, 512], BF16)
            nc.vector.memset(warm_rhs, 0.0)
            ones_bf = const.tile([P, 1], BF16)
            nc.vector.memset(ones_bf, 1.0)

            # --- PE warmup: release the HAM clock gate before real matmuls
            warm_pt = psum2.tile([P, 2048], F32, tag="pt2")
            for w in range(WARM_MMS):
                off = (w % 4) * 512
                nc.tensor.matmul(
                    warm_pt[:1, off : off + 512], lhsT=ones_bf, rhs=warm_rhs,
                    start=True, stop=True,
                )

            def rsqrt_dve(a, y, w):
                """y[:,:w] = 1/sqrt(a[:,:w]): quake seed + 1 Newton step."""
                au = a.bitcast(U32)
                yu = y.bitcast(U32)
                sh = work.tile([P, GCH], U32, tag="rsq_sh")
                nc.vector.tensor_scalar(
                    out=sh[:, :w], in0=au, scalar1=1, scalar2=None,
                    op0=ALU.logical_shift_right,
                )
                nc.vector.tensor_sub(out=yu, in0=magic[:, :w], in1=sh[:, :w])
                t1 = work.tile([P, GCH], F32, tag="rsq_t1")
                nc.vector.tensor_mul(out=t1[:, :w], in0=y, in1=y)
                nc.vector.tensor_mul(out=t1[:, :w], in0=t1[:, :w], in1=a)
                nc.vector.tensor_scalar(
                    out=t1[:, :w], in0=t1[:, :w], scalar1=-0.5, scalar2=1.5,
                    op0=ALU.mult, op1=ALU.add,
                )
                nc.vector.tensor_mul(out=y, in0=y, in1=t1[:, :w])

            # --- loads (ACT HWDGE queue; all in the prologue)
            zi_bf = big.tile([P, NCHUNK, D], BF16)
            nc.scalar.dma_start(
                out=zi_bf, in_=z_i.rearrange("(c p) d -> p c d", p=P)
            )
            zj_f = []
            for g in range(SG):
                t = big.tile([P, GCH, D], BF16, tag=f"zjf{g}")
                nc.scalar.dma_start(
                    out=t,
                    in_=z_j[g * SGM : (g + 1) * SGM, :].rearrange(
                        "(c p) d -> p c d", p=P
                    ),
                )
                zj_f.append(t)
            zjd_f = big.tile([P, NCHUNK, D], BF16)
            nc.scalar.dma_start(
                out=zjd_f, in_=z_jd.rearrange("(c p) d -> p c d", p=P)
            )

            ziT = big.tile([P, NCHUNK * KH, P], BF16)
            nc.sync.dma_start_transpose(
                out=ziT, in_=zi_bf.rearrange("p c d -> p (c d)")
            )
            ziT_r = ziT.rearrange("do (i h) m -> do i h m", h=KH)

            # --- zi norms on DVE (wide), s2 = 2*rsqrt(nrm2)
            nrm2_i = stat.tile([P, NCHUNK], F32)
            sqi = work.tile([P, NCHUNK, D], BF16, tag="sqd")
            nc.vector.tensor_mul(out=sqi, in0=zi_bf, in1=zi_bf)
            nc.vector.tensor_reduce(
                out=nrm2_i, in_=sqi, axis=AX.X, op=ALU.add
            )
            t_i = stat.tile([P, NCHUNK], F32)
            rsqrt_dve(nrm2_i, t_i, NCHUNK)
            s2 = stat.tile([P, NCHUNK], F32)
            nc.vector.tensor_scalar(
                out=s2, in0=t_i, scalar1=2.0, scalar2=None, op0=ALU.mult
            )

            # --- per-subgroup zj prep (all DVE + sync-queue transpose)
            nrm2_j = stat.tile([P, SG * GCH], F32)
            t_j = stat.tile([P, SG * GCH], F32)
            zjT_r = []

            def zj_group(g):
                gs = slice(g * GCH, (g + 1) * GCH)
                sqw = work.tile([P, GCH, D], BF16, tag="sqw")
                nc.vector.tensor_mul(out=sqw, in0=zj_f[g], in1=zj_f[g])
                nc.vector.tensor_reduce(
                    out=nrm2_j[:, gs], in_=sqw, axis=AX.X, op=ALU.add
                )
                rsqrt_dve(nrm2_j[:, gs], t_j[:, gs], GCH)
                zjs = big.tile([P, GCH, D], BF16, tag=f"zjs{g}")
                for jl in range(GCH):
                    j = g * GCH + jl
                    nc.vector.tensor_scalar_mul(
                        out=zjs[:, jl, :],
                        in0=zj_f[g][:, jl, :],
                        scalar1=t_j[:, j : j + 1],
                    )
                zjT = big.tile([P, GCH * KH, P], BF16, tag=f"zjT{g}")
                nc.sync.dma_start_transpose(
                    out=zjT, in_=zjs.rearrange("p c d -> p (c d)")
                )
                zjT_r.append(zjT.rearrange("do (c h) m -> do c h m", h=KH))

            for g in range(SG):
                zj_group(g)

            # --- compute waves ---------------------------------------------
            # wave A: sg0 as 4 single tiles; wave B: (sg1,sg2) paired;
            # wave C: sg3 as 4 single tiles.
            lse_parts = stat.tile([P, 3 * NCHUNK], F32)

            def single_tile(g, i, k):
                pt = psum1.tile([P, 1024], F32, tag="pt1")
                for h in range(KH):
                    for jj in range(2):
                        nc.tensor.matmul(
                            pt[:, jj * 512 : (jj + 1) * 512],
                            lhsT=ziT_r[:, i, h, :],
                            rhs=zjT_r[g][:, jj * 4 : jj * 4 + 4, h, :],
                            start=(h == 0),
                            stop=(h == KH - 1),
                        )
                nc.scalar.activation(
                    out=pt, in_=pt, func=AF.Exp,
                    scale=s2[:, i : i + 1],
                    accum_out=lse_parts[:, k : k + 1],
                )

            def pair_tile(i, k):
                pt = psum2.tile([P, 2048], F32, tag="pt2")
                for half in range(2):
                    g = 1 + half
                    for h in range(KH):
                        for jj in range(2):
                            off = half * 1024 + jj * 512
                            nc.tensor.matmul(
                                pt[:, off : off + 512],
                                lhsT=ziT_r[:, i, h, :],
                                rhs=zjT_r[g][:, jj * 4 : jj * 4 + 4, h, :],
                                start=(h == 0),
                                stop=(h == KH - 1),
                            )
                nc.scalar.activation(
                    out=pt, in_=pt, func=AF.Exp,
                    scale=s2[:, i : i + 1],
                    accum_out=lse_parts[:, k : k + 1],
                )

            for i in range(NCHUNK):
                single_tile(0, i, i)

            # --- zjd squares on ScalarE: fills the ACT bubble between waves
            nrm2_d = stat.tile([P, NCHUNK], F32)
            junk_act = work.tile([P, D], BF16, tag="junk_act")
            for i in range(NCHUNK):
                nc.scalar.activation(
                    out=junk_act, in_=zjd_f[:, i, :], func=AF.Square,
                    accum_out=nrm2_d[:, i : i + 1],
                )

            for i in range(NCHUNK):
                pair_tile(i, NCHUNK + i)
            for i in range(NCHUNK):
                single_tile(3, i, 2 * NCHUNK + i)

            # --- diag: dt = sum(zi*zjd); dg = dt * t_d * s2 (DVE)
            t_d = stat.tile([P, NCHUNK], F32)
            rsqrt_dve(nrm2_d, t_d, NCHUNK)
            dt = stat.tile([P, NCHUNK], F32)
            dprod = work.tile([P, NCHUNK, D], BF16, tag="sqd")
            nc.vector.tensor_mul(out=dprod, in0=zi_bf, in1=zjd_f)
            nc.vector.tensor_reduce(
                out=dt, in_=dprod, axis=AX.X, op=ALU.add
            )
            dg0 = stat.tile([P, NCHUNK], F32)
            nc.vector.tensor_mul(out=dg0, in0=dt, in1=t_d)
            dg = stat.tile([P, NCHUNK], F32)
            nc.vector.tensor_mul(out=dg, in0=dg0, in1=s2)

            # --- lse = ln(S) via bit-trick + one Newton refinement
            rs0 = stat.tile([P, NCHUNK], F32)
            nc.vector.tensor_add(
                out=rs0, in0=lse_parts[:, :NCHUNK],
                in1=lse_parts[:, NCHUNK : 2 * NCHUNK],
            )
            rs = stat.tile([P, NCHUNK], F32)
            nc.vector.tensor_add(
                out=rs, in0=rs0, in1=lse_parts[:, 2 * NCHUNK :]
            )
            vf = stat.tile([P, NCHUNK], F32)
            nc.vector.tensor_copy(out=vf, in_=rs.bitcast(I32))
            lse0 = stat.tile([P, NCHUNK], F32)
            nc.vector.tensor_scalar(
                out=lse0, in0=vf, scalar1=LN_B, scalar2=LN_S,
                op0=ALU.subtract, op1=ALU.mult,
            )
            ef = stat.tile([P, NCHUNK], F32)
            nc.scalar.activation(out=ef, in_=lse0, func=AF.Exp, scale=-1.0)
            rf = stat.tile([P, NCHUNK], F32)
            nc.vector.tensor_mul(out=rf, in0=rs, in1=ef)
            uf = stat.tile([P, NCHUNK], F32)
            nc.vector.tensor_scalar(
                out=uf, in0=rf, scalar1=1.0, scalar2=None, op0=ALU.subtract
            )
            qf = stat.tile([P, NCHUNK], F32)
            nc.vector.scalar_tensor_tensor(
                out=qf, in0=uf, scalar=-0.5, in1=uf,
                op0=ALU.mult, op1=ALU.mult,
            )
            l1 = stat.tile([P, NCHUNK], F32)
            nc.vector.tensor_add(out=l1, in0=lse0, in1=uf)
            lse = stat.tile([P, NCHUNK], F32)
            nc.vector.tensor_add(out=lse, in0=l1, in1=qf)
            contrib = stat.tile([P, NCHUNK], F32)
            nc.vector.tensor_sub(out=contrib, in0=lse, in1=dg)

            # --- partition reduction via ones-matmul: [1, 4] partials
            pt_fin = psum1.tile([P, 1024], F32, tag="pt1")
            nc.tensor.matmul(
                pt_fin[:1, :NCHUNK], lhsT=ones, rhs=contrib, start=True, stop=True
            )
            osb = stat.tile([1, NCHUNK], F32)
            nc.vector.tensor_copy(out=osb, in_=pt_fin[:1, :NCHUNK])
            nc.sync.dma_start(out=out, in_=osb)

    nc.compile()
    return nc


_NC = None


def _get_nc():
    global _NC
    if _NC is None:
        _NC = build_nc()
    return _NC


def make_in_maps(z_i: np.ndarray, z_j: np.ndarray):
    z_i = np.asarray(z_i).astype(ml_dtypes.bfloat16)
    z_j = np.asarray(z_j).astype(ml_dtypes.bfloat16)
    in_maps = []
    for c in range(NCORES):
        sl = slice(c * NL, (c + 1) * NL)
        in_maps.append(
            {
                "z_i": np.ascontiguousarray(z_i[sl]),
                "z_j": z_j,
                "z_jd": np.ascontiguousarray(z_j[sl]),
            }
        )
    return in_maps


def kernel(z_i: np.ndarray, z_j: np.ndarray, **_unused) -> np.ndarray:
    nc = _get_nc()
    in_maps = make_in_maps(z_i, z_j)
    res = bass_utils.run_bass_kernel_spmd(
        nc, in_maps, core_ids=list(range(NCORES))
    )
    total = 0.0
    for c in range(NCORES):
        total += float(res.results[c]["out"].astype(np.float64).sum())
    return np.float32(total / N)


# revision 11
# speedup vs baseline: 2.5548x; 1.1438x over previous
"""Contrastive loss (SimCLR-style) on 8 TRN2 NeuronCores.

loss = -mean(diag(log_softmax(zi_n @ zj_n^T / T)))  with zi_n, zj_n L2-normalized,
N=4096, D=256, T=0.5.

v5 design (data-parallel over rows of z_i, z_j replicated):
  - host casts all inputs to bf16.
  - z_i unnormalized; row norm folds into the exp per-partition scale
    (s2 = 2/|zi|).
  - z_j in 4 subgroups of 1024 rows; per sg on DVE: wide bf16 square (2x)
    + one tensor_reduce + seed-only rsqrt bit-trick + 8 per-chunk scales,
    then bf16 xbar transpose (sync queue).  DVE program order is pinned
    with explicit nosync dep edges so the scheduler cannot interleave
    subgroups (which would stall the whole FIFO on a late load).
  - loads are issued before any transpose (transposes serialize against
    in-flight DMA in HW): z_i+sg0 on the sync queue, sg1-3+zjd on the
    ACT queue; the exp table load comes after the loads.
  - logits: 16 single [128,1024] PSUM tiles (4 banks x 4 bufs), 4 matmuls
    + one fused exp/accum ScalarE activation per tile, in-place over PSUM.
  - a few warm matmuls pinned right before the first real one keep the PE
    HAM clock gate released.
  - zjd diag block entirely on DVE, after the sg chains (fills DVE idle
    under the exp stream).
  - lse's ln via DVE bit-trick + one exp-based Newton refinement.
  - each core returns 4 partial sums of (lse[n] - diag[n]); host adds the
    32 values and divides by N.
"""

import numpy as np
import ml_dtypes

import concourse.bass as bass
import concourse.bacc as bacc
import concourse.tile as tile
import concourse.bass_utils as bass_utils
from concourse import mybir
from concourse.tile_rust import add_dep_helper

N = 4096
D = 256
NCORES = 8
NL = N // NCORES  # 512 local rows per core
P = 128
NCHUNK = NL // P  # 4 local row chunks
SG = 4  # zj subgroups
SGM = N // SG  # 1024 rows per subgroup
GCH = SGM // P  # 8 chunks per subgroup
KH = D // P  # 2 contraction halves
MAGIC = 0x5F3759DF

LN_B = 1064872509.0
LN_S = 0.6931471805599453 / 8388608.0  # ln2 / 2^23

WARM_MMS = 4

F32 = mybir.dt.float32
I32 = mybir.dt.int32
U32 = mybir.dt.uint32
BF16 = mybir.dt.bfloat16
AF = mybir.ActivationFunctionType
ALU = mybir.AluOpType
AX = mybir.AxisListType


def build_nc():
    nc = bacc.Bacc(
        "TRN2",
        target_bir_lowering=False,
        debug=False,
        enable_asserts=False,
    )
    z_i = nc.dram_tensor("z_i", (NL, D), BF16, kind="ExternalInput").ap()
    z_j = nc.dram_tensor("z_j", (N, D), BF16, kind="ExternalInput").ap()
    z_jd = nc.dram_tensor("z_jd", (NL, D), BF16, kind="ExternalInput").ap()
    out = nc.dram_tensor("out", (1, NCHUNK), F32, kind="ExternalOutput").ap()

    with tile.TileContext(nc) as tc:
        with (
            tc.tile_pool(name="const", bufs=1) as const,
            tc.tile_pool(name="big", bufs=1) as big,
            tc.tile_pool(name="work", bufs=2) as work,
            tc.tile_pool(name="stat", bufs=1) as stat,
            tc.tile_pool(name="psum", bufs=4, space="PSUM") as psum,
        ):
            # --- constants
            dummy = const.tile([1, 1], F32)
            nc.vector.memset(dummy, 1.0)
            ones = const.tile([P, 1], F32)
            nc.vector.memset(ones, 1.0)
            magic = const.tile([P, GCH], U32)
            nc.vector.memset(magic, MAGIC)
            warm_rhs = const.tile([P, 512], BF16)
            nc.vector.memset(warm_rhs, 0.0)
            ones_bf = const.tile([P, 1], BF16)
            nc.vector.memset(ones_bf, 1.0)

            # --- loads first (before any transpose): zi+sg0 on sync,
            #     sg1-3+zjd on the ACT hwdge queue
            zi_bf = big.tile([P, NCHUNK, D], BF16)
            nc.sync.dma_start(
                out=zi_bf, in_=z_i.rearrange("(c p) d -> p c d", p=P)
            )
            zj_f = []
            for g in range(SG):
                t = big.tile([P, GCH, D], BF16, tag=f"zjf{g}")
                eng = nc.sync if g == 0 else nc.scalar
                eng.dma_start(
                    out=t,
                    in_=z_j[g * SGM : (g + 1) * SGM, :].rearrange(
                        "(c p) d -> p c d", p=P
                    ),
                )
                zj_f.append(t)
            zjd_f = big.tile([P, NCHUNK, D], BF16)
            nc.scalar.dma_start(
                out=zjd_f, in_=z_jd.rearrange("(c p) d -> p c d", p=P)
            )

            # exp table load while preprocessing runs
            nc.scalar.activation(out=dummy, in_=dummy, func=AF.Exp)

            ziT = big.tile([P, NCHUNK * KH, P], BF16)
            nc.sync.dma_start_transpose(
                out=ziT, in_=zi_bf.rearrange("p c d -> p (c d)")
            )
            ziT_r = ziT.rearrange("do (i h) m -> do i h m", h=KH)

            # --- pinned DVE ordering helper
            last_dve = [None]

            def dve(bi):
                if last_dve[0] is not None:
                    add_dep_helper(
                        bi.ins, last_dve[0], sync=False, reason="dve order"
                    )
                last_dve[0] = bi.ins
                return bi

            def rsqrt_full(a, y, w):
                """y = 1/sqrt(a): quake seed + 1 Newton step (6 DVE ops)."""
                au = a.bitcast(U32)
                yu = y.bitcast(U32)
                sh = work.tile([P, GCH], U32, tag="rsq_sh")
                dve(nc.vector.tensor_scalar(
                    out=sh[:, :w], in0=au, scalar1=1, scalar2=None,
                    op0=ALU.logical_shift_right,
                ))
                dve(nc.vector.tensor_sub(out=yu, in0=magic[:, :w], in1=sh[:, :w]))
                t1 = work.tile([P, GCH], F32, tag="rsq_t1")
                dve(nc.vector.tensor_mul(out=t1[:, :w], in0=y, in1=y))
                dve(nc.vector.tensor_mul(out=t1[:, :w], in0=t1[:, :w], in1=a))
                dve(nc.vector.tensor_scalar(
                    out=t1[:, :w], in0=t1[:, :w], scalar1=-0.5, scalar2=1.5,
                    op0=ALU.mult, op1=ALU.add,
                ))
                dve(nc.vector.tensor_mul(out=y, in0=y, in1=t1[:, :w]))

            def rsqrt_seed(a, y, w):
                """y ~= 1/sqrt(a): quake seed only (2 DVE ops, ~3% err)."""
                au = a.bitcast(U32)
                yu = y.bitcast(U32)
                sh = work.tile([P, GCH], U32, tag="rsq_sh")
                dve(nc.vector.tensor_scalar(
                    out=sh[:, :w], in0=au, scalar1=1, scalar2=None,
                    op0=ALU.logical_shift_right,
                ))
                dve(nc.vector.tensor_sub(out=yu, in0=magic[:, :w], in1=sh[:, :w]))

            # --- zi norms (DVE, wide), s2 = 2*rsqrt(nrm2)
            nrm2_i = stat.tile([P, NCHUNK], F32)
            sqi = work.tile([P, NCHUNK, D], BF16, tag="sqd")
            dve(nc.vector.tensor_mul(out=sqi, in0=zi_bf, in1=zi_bf))
            dve(nc.vector.tensor_reduce(
                out=nrm2_i, in_=sqi, axis=AX.X, op=ALU.add
            ))
            t_i = stat.tile([P, NCHUNK], F32)
            rsqrt_full(nrm2_i, t_i, NCHUNK)
            s2 = stat.tile([P, NCHUNK], F32)
            dve(nc.vector.tensor_scalar(
                out=s2, in0=t_i, scalar1=2.0, scalar2=None, op0=ALU.mult
            ))

            # --- per-subgroup zj prep (DVE chain + sync-queue transpose)
            nrm2_j = stat.tile([P, SG * GCH], F32)
            t_j = stat.tile([P, SG * GCH], F32)
            zjT_r = []
            sg_sq_ins = []

            def zj_group(g):
                gs = slice(g * GCH, (g + 1) * GCH)
                sqw = work.tile([P, GCH, D], BF16, tag="sqw")
                bi = dve(nc.vector.tensor_mul(out=sqw, in0=zj_f[g], in1=zj_f[g]))
                sg_sq_ins.append(bi.ins)
                dve(nc.vector.tensor_reduce(
                    out=nrm2_j[:, gs], in_=sqw, axis=AX.X, op=ALU.add
                ))
                rsqrt_seed(nrm2_j[:, gs], t_j[:, gs], GCH)
                zjs = big.tile([P, GCH, D], BF16, tag=f"zjs{g}")
                for jl in range(GCH):
                    j = g * GCH + jl
                    dve(nc.vector.tensor_scalar_mul(
                        out=zjs[:, jl, :],
                        in0=zj_f[g][:, jl, :],
                        scalar1=t_j[:, j : j + 1],
                    ))
                zjT = big.tile([P, GCH * KH, P], BF16, tag=f"zjT{g}")
                nc.sync.dma_start_transpose(
                    out=zjT, in_=zjs.rearrange("p c d -> p (c d)")
                )
                zjT_r.append(zjT.rearrange("do (c h) m -> do c h m", h=KH))

            for g in range(SG):
                zj_group(g)

            # --- warm matmuls: anchored to sg0's square so they run just
            #     before the first real matmul and keep HAM released
            warm_pt = psum.tile([P, 1024], F32, tag="pt")
            for w in range(WARM_MMS):
                off = (w % 2) * 512
                mm = nc.tensor.matmul(
                    warm_pt[:1, off : off + 512], lhsT=ones_bf, rhs=warm_rhs,
                    start=True, stop=True,
                )
                if w == 0:
                    add_dep_helper(
                        mm.ins, sg_sq_ins[0], sync=False, reason="warm anchor"
                    )

            # --- compute: 16 single [128,1024] tiles, fused exp on ScalarE
            lse_parts = stat.tile([P, SG * NCHUNK], F32)

            def logits_tile(g, i):
                k = g * NCHUNK + i
                pt = psum.tile([P, 1024], F32, tag="pt")
                for h in range(KH):
                    for jj in range(2):
                        nc.tensor.matmul(
                            pt[:, jj * 512 : (jj + 1) * 512],
                            lhsT=ziT_r[:, i, h, :],
                            rhs=zjT_r[g][:, jj * 4 : jj * 4 + 4, h, :],
                            start=(h == 0),
                            stop=(h == KH - 1),
                        )
                nc.scalar.activation(
                    out=pt, in_=pt, func=AF.Exp,
                    scale=s2[:, i : i + 1],
                    accum_out=lse_parts[:, k : k + 1],
                )

            for g in range(SG):
                for i in range(NCHUNK):
                    logits_tile(g, i)

            # --- zjd diag block on DVE (fills idle under the exp stream)
            nrm2_d = stat.tile([P, NCHUNK], F32)
            sqd = work.tile([P, NCHUNK, D], BF16, tag="sqd")
            dve(nc.vector.tensor_mul(out=sqd, in0=zjd_f, in1=zjd_f))
            dve(nc.vector.tensor_reduce(
                out=nrm2_d, in_=sqd, axis=AX.X, op=ALU.add
            ))
            t_d = stat.tile([P, NCHUNK], F32)
            rsqrt_full(nrm2_d, t_d, NCHUNK)
            dt = stat.tile([P, NCHUNK], F32)
            dprod = work.tile([P, NCHUNK, D], BF16, tag="sqd")
            dve(nc.vector.tensor_mul(out=dprod, in0=zi_bf, in1=zjd_f))
            dve(nc.vector.tensor_reduce(
                out=dt, in_=dprod, axis=AX.X, op=ALU.add
            ))
            dg0 = stat.tile([P, NCHUNK], F32)
            dve(nc.vector.tensor_mul(out=dg0, in0=dt, in1=t_d))
            dg = stat.tile([P, NCHUNK], F32)
            dve(nc.vector.tensor_mul(out=dg, in0=dg0, in1=s2))

            # --- lse = ln(S) via bit-trick + one Newton refinement
            rs0 = stat.tile([P, NCHUNK], F32)
            dve(nc.vector.tensor_add(
                out=rs0, in0=lse_parts[:, :NCHUNK],
                in1=lse_parts[:, NCHUNK : 2 * NCHUNK],
            ))
            rs1 = stat.tile([P, NCHUNK], F32)
            dve(nc.vector.tensor_add(
                out=rs1, in0=lse_parts[:, 2 * NCHUNK : 3 * NCHUNK],
                in1=lse_parts[:, 3 * NCHUNK :],
            ))
            rs = stat.tile([P, NCHUNK], F32)
            dve(nc.vector.tensor_add(out=rs, in0=rs0, in1=rs1))
            vf = stat.tile([P, NCHUNK], F32)
            dve(nc.vector.tensor_copy(out=vf, in_=rs.bitcast(I32)))
            lse0 = stat.tile([P, NCHUNK], F32)
            dve(nc.vector.tensor_scalar(
                out=lse0, in0=vf, scalar1=LN_B, scalar2=LN_S,
                op0=ALU.subtract, op1=ALU.mult,
            ))
            ef = stat.tile([P, NCHUNK], F32)
            nc.scalar.activation(out=ef, in_=lse0, func=AF.Exp, scale=-1.0)
            rf = stat.tile([P, NCHUNK], F32)
            dve(nc.vector.tensor_mul(out=rf, in0=rs, in1=ef))
            uf = stat.tile([P, NCHUNK], F32)
            dve(nc.vector.tensor_scalar(
                out=uf, in0=rf, scalar1=1.0, scalar2=None, op0=ALU.subtract
            ))
            qf = stat.tile([P, NCHUNK], F32)
            dve(nc.vector.scalar_tensor_tensor(
                out=qf, in0=uf, scalar=-0.5, in1=uf,
                op0=ALU.mult, op1=ALU.mult,
            ))
            l1 = stat.tile([P, NCHUNK], F32)
            dve(nc.vector.tensor_add(out=l1, in0=lse0, in1=uf))
            lse = stat.tile([P, NCHUNK], F32)
            dve(nc.vector.tensor_add(out=lse, in0=l1, in1=qf))
            contrib = stat.tile([P, NCHUNK], F32)
            dve(nc.vector.tensor_sub(out=contrib, in0=lse, in1=dg))

            # --- partition reduction via ones-matmul: [1, 4] partials
            pt_fin = psum.tile([P, 1024], F32, tag="pt")
            nc.tensor.matmul(
                pt_fin[:1, :NCHUNK], lhsT=ones, rhs=contrib, start=True, stop=True
            )
            osb = stat.tile([1, NCHUNK], F32)
            nc.vector.tensor_copy(out=osb, in_=pt_fin[:1, :NCHUNK])
            nc.sync.dma_start(out=out, in_=osb)

    nc.compile()
    return nc


_NC = None


def _get_nc():
    global _NC
    if _NC is None:
        _NC = build_nc()
    return _NC


def make_in_maps(z_i: np.ndarray, z_j: np.ndarray):
    z_i = np.asarray(z_i).astype(ml_dtypes.bfloat16)
    z_j = np.asarray(z_j).astype(ml_dtypes.bfloat16)
    in_maps = []
    for c in range(NCORES):
        sl = slice(c * NL, (c + 1) * NL)
        in_maps.append(
            {
                "z_i": np.ascontiguousarray(z_i[sl]),
                "z_j": z_j,
                "z_jd": np.ascontiguousarray(z_j[sl]),
            }
        )
    return in_maps


def kernel(z_i: np.ndarray, z_j: np.ndarray, **_unused) -> np.ndarray:
    nc = _get_nc()
    in_maps = make_in_maps(z_i, z_j)
    res = bass_utils.run_bass_kernel_spmd(
        nc, in_maps, core_ids=list(range(NCORES))
    )
    total = 0.0
    for c in range(NCORES):
        total += float(res.results[c]["out"].astype(np.float64).sum())
    return np.float32(total / N)


# revision 17
# speedup vs baseline: 2.6860x; 1.0513x over previous
"""Contrastive loss (SimCLR-style) on 8 TRN2 NeuronCores.

loss = -mean(diag(log_softmax(zi_n @ zj_n^T / T)))  with zi_n, zj_n L2-normalized,
N=4096, D=256, T=0.5.

v5 design (data-parallel over rows of z_i, z_j replicated):
  - host casts all inputs to bf16.
  - z_i unnormalized; row norm folds into the exp per-partition scale
    (s2 = 2/|zi|).
  - z_j in 4 subgroups of 1024 rows; per sg on DVE: wide bf16 square (2x)
    + one tensor_reduce + seed-only rsqrt bit-trick + 8 per-chunk scales,
    then bf16 xbar transpose (sync queue).  DVE program order is pinned
    with explicit nosync dep edges so the scheduler cannot interleave
    subgroups (which would stall the whole FIFO on a late load).
  - loads are issued before any transpose (transposes serialize against
    in-flight DMA in HW): z_i+sg0 on the sync queue, sg1-3+zjd on the
    ACT queue; the exp table load comes after the loads.
  - logits: 16 single [128,1024] PSUM tiles (4 banks x 4 bufs), 4 matmuls
    + one fused exp/accum ScalarE activation per tile, in-place over PSUM.
  - a few warm matmuls pinned right before the first real one keep the PE
    HAM clock gate released.
  - zjd diag block entirely on DVE, after the sg chains (fills DVE idle
    under the exp stream).
  - lse's ln via DVE bit-trick + one exp-based Newton refinement.
  - each core returns 4 partial sums of (lse[n] - diag[n]); host adds the
    32 values and divides by N.
"""

import numpy as np
import ml_dtypes

import concourse.bass as bass
import concourse.bacc as bacc
import concourse.tile as tile
import concourse.bass_utils as bass_utils
from concourse import mybir
from concourse.tile_rust import add_dep_helper

N = 4096
D = 256
NCORES = 8
NL = N // NCORES  # 512 local rows per core
P = 128
NCHUNK = NL // P  # 4 local row chunks
SG = 4  # zj subgroups
SGM = N // SG  # 1024 rows per subgroup
GCH = SGM // P  # 8 chunks per subgroup
KH = D // P  # 2 contraction halves
MAGIC = 0x5F3759DF

LN_B = 1064872509.0
LN_S = 0.6931471805599453 / 8388608.0  # ln2 / 2^23

WARM_MMS = 4

F32 = mybir.dt.float32
I32 = mybir.dt.int32
U32 = mybir.dt.uint32
BF16 = mybir.dt.bfloat16
AF = mybir.ActivationFunctionType
ALU = mybir.AluOpType
AX = mybir.AxisListType


def build_nc():
    nc = bacc.Bacc(
        "TRN2",
        target_bir_lowering=False,
        debug=False,
        enable_asserts=False,
    )
    z_i = nc.dram_tensor("z_i", (NL, D), BF16, kind="ExternalInput").ap()
    z_j = nc.dram_tensor("z_j", (N, D), BF16, kind="ExternalInput").ap()
    out = nc.dram_tensor("out", (1, NCHUNK), F32, kind="ExternalOutput").ap()

    with tile.TileContext(nc) as tc:
        with (
            tc.tile_pool(name="const", bufs=1) as const,
            tc.tile_pool(name="big", bufs=1) as big,
            tc.tile_pool(name="work", bufs=2) as work,
            tc.tile_pool(name="stat", bufs=1) as stat,
            tc.tile_pool(name="psum", bufs=4, space="PSUM") as psum,
        ):
            # --- constants
            dummy = const.tile([1, 1], F32)
            nc.vector.memset(dummy, 1.0)
            ones = const.tile([P, 1], F32)
            nc.vector.memset(ones, 1.0)
            magic = const.tile([P, GCH], U32)
            nc.vector.memset(magic, MAGIC)
            warm_rhs = const.tile([P, 512], BF16)
            nc.vector.memset(warm_rhs, 0.0)
            ones_bf = const.tile([P, 1], BF16)
            nc.vector.memset(ones_bf, 1.0)

            # --- loads first (before any transpose): zi+sg0 on sync,
            #     sg1-3 on the ACT hwdge queue.  z_j arrives host-rotated so
            #     rows 0-511 are this core's own diagonal block.
            zi_bf = big.tile([P, NCHUNK, D], BF16)
            nc.sync.dma_start(
                out=zi_bf, in_=z_i.rearrange("(c p) d -> p c d", p=P)
            )
            zj_f = []
            for g in range(SG):
                t = big.tile([P, GCH, D], BF16, tag=f"zjf{g}")
                eng = nc.sync if g == 0 else nc.scalar
                eng.dma_start(
                    out=t,
                    in_=z_j[g * SGM : (g + 1) * SGM, :].rearrange(
                        "(c p) d -> p c d", p=P
                    ),
                )
                zj_f.append(t)

            # exp table load while preprocessing runs
            nc.scalar.activation(out=dummy, in_=dummy, func=AF.Exp)

            # ziT transpose on the ACT queue: runs after the scalar-queue
            # loads drain, well before the first matmul needs it; keeps the
            # sync queue free for the sg transposes.
            ziT = big.tile([P, NCHUNK * KH, P], BF16)
            nc.scalar.dma_start_transpose(
                out=ziT, in_=zi_bf.rearrange("p c d -> p (c d)")
            )
            ziT_r = ziT.rearrange("do (i h) m -> do i h m", h=KH)

            # --- pinned DVE ordering helper
            last_dve = [None]

            def dve(bi):
                if last_dve[0] is not None:
                    add_dep_helper(
                        bi.ins, last_dve[0], sync=False, reason="dve order"
                    )
                last_dve[0] = bi.ins
                return bi

            def rsqrt_full(a, y, w):
                """y = 1/sqrt(a): quake seed + 1 Newton step (6 DVE ops)."""
                au = a.bitcast(U32)
                yu = y.bitcast(U32)
                sh = work.tile([P, GCH], U32, tag="rsq_sh")
                dve(nc.vector.tensor_scalar(
                    out=sh[:, :w], in0=au, scalar1=1, scalar2=None,
                    op0=ALU.logical_shift_right,
                ))
                dve(nc.vector.tensor_sub(out=yu, in0=magic[:, :w], in1=sh[:, :w]))
                t1 = work.tile([P, GCH], F32, tag="rsq_t1")
                dve(nc.vector.tensor_mul(out=t1[:, :w], in0=y, in1=y))
                dve(nc.vector.tensor_mul(out=t1[:, :w], in0=t1[:, :w], in1=a))
                dve(nc.vector.tensor_scalar(
                    out=t1[:, :w], in0=t1[:, :w], scalar1=-0.5, scalar2=1.5,
                    op0=ALU.mult, op1=ALU.add,
                ))
                dve(nc.vector.tensor_mul(out=y, in0=y, in1=t1[:, :w]))

            def rsqrt_seed(a, y, w):
                """y ~= 1/sqrt(a): quake seed only (2 DVE ops, ~3% err)."""
                au = a.bitcast(U32)
                yu = y.bitcast(U32)
                sh = work.tile([P, GCH], U32, tag="rsq_sh")
                dve(nc.vector.tensor_scalar(
                    out=sh[:, :w], in0=au, scalar1=1, scalar2=None,
                    op0=ALU.logical_shift_right,
                ))
                dve(nc.vector.tensor_sub(out=yu, in0=magic[:, :w], in1=sh[:, :w]))

            # --- zi norms (DVE, wide), s2 = 2*rsqrt(nrm2)
            nrm2_i = stat.tile([P, NCHUNK], F32)
            sqi = work.tile([P, NCHUNK, D], BF16, tag="sqd")
            dve(nc.vector.tensor_mul(out=sqi, in0=zi_bf, in1=zi_bf))
            dve(nc.vector.tensor_reduce(
                out=nrm2_i, in_=sqi, axis=AX.X, op=ALU.add
            ))
            t_i = stat.tile([P, NCHUNK], F32)
            rsqrt_full(nrm2_i, t_i, NCHUNK)
            s2 = stat.tile([P, NCHUNK], F32)
            dve(nc.vector.tensor_scalar(
                out=s2, in0=t_i, scalar1=2.0, scalar2=None, op0=ALU.mult
            ))

            # --- per-subgroup zj prep (DVE chain + sync-queue transpose)
            nrm2_j = stat.tile([P, SG * GCH], F32)
            t_j = stat.tile([P, SG * GCH], F32)
            zjT_r = []
            sg_sq_ins = []

            def zj_group(g):
                gs = slice(g * GCH, (g + 1) * GCH)
                sqw = work.tile([P, GCH, D], BF16, tag="sqw")
                bi = dve(nc.vector.tensor_mul(out=sqw, in0=zj_f[g], in1=zj_f[g]))
                sg_sq_ins.append(bi.ins)
                fold = work.tile([P, GCH, P], BF16, tag="fold")
                dve(nc.vector.tensor_add(
                    out=fold, in0=sqw[:, :, :P], in1=sqw[:, :, P:]
                ))
                dve(nc.vector.tensor_reduce(
                    out=nrm2_j[:, gs], in_=fold, axis=AX.X, op=ALU.add
                ))
                rsqrt_seed(nrm2_j[:, gs], t_j[:, gs], GCH)
                zjs = big.tile([P, GCH, D], BF16, tag=f"zjs{g}")
                for jl in range(GCH):
                    j = g * GCH + jl
                    dve(nc.vector.tensor_scalar_mul(
                        out=zjs[:, jl, :],
                        in0=zj_f[g][:, jl, :],
                        scalar1=t_j[:, j : j + 1],
                    ))
                zjT = big.tile([P, GCH * KH, P], BF16, tag=f"zjT{g}")
                nc.sync.dma_start_transpose(
                    out=zjT, in_=zjs.rearrange("p c d -> p (c d)")
                )
                zjT_r.append(zjT.rearrange("do (c h) m -> do c h m", h=KH))

            for g in range(SG):
                zj_group(g)

            # --- warm matmuls: anchored to sg0's square so they run just
            #     before the first real matmul and keep HAM released
            warm_pt = psum.tile([P, 1024], F32, tag="pt")
            for w in range(WARM_MMS):
                off = (w % 2) * 512
                mm = nc.tensor.matmul(
                    warm_pt[:1, off : off + 512], lhsT=ones_bf, rhs=warm_rhs,
                    start=True, stop=True,
                )
                if w == 0:
                    add_dep_helper(
                        mm.ins, sg_sq_ins[0], sync=True, reason="warm anchor"
                    )

            # --- compute: 16 single [128,1024] tiles, fused exp on ScalarE
            lse_parts = stat.tile([P, SG * NCHUNK], F32)

            def logits_tile(g, i):
                k = g * NCHUNK + i
                pt = psum.tile([P, 1024], F32, tag="pt")
                for h in range(KH):
                    for jj in range(2):
                        nc.tensor.matmul(
                            pt[:, jj * 512 : (jj + 1) * 512],
                            lhsT=ziT_r[:, i, h, :],
                            rhs=zjT_r[g][:, jj * 4 : jj * 4 + 4, h, :],
                            start=(h == 0),
                            stop=(h == KH - 1),
                        )
                nc.scalar.activation(
                    out=pt, in_=pt, func=AF.Exp,
                    scale=s2[:, i : i + 1],
                    accum_out=lse_parts[:, k : k + 1],
                )

            for g in range(SG):
                for i in range(NCHUNK):
                    logits_tile(g, i)

            # --- diag block from the rotated z_j's first 4 chunks (rows
            #     0-511 = this core's own rows); accurate rsqrt for t_d.
            t_d = stat.tile([P, NCHUNK], F32)
            rsqrt_full(nrm2_j[:, :NCHUNK], t_d, NCHUNK)
            dt = stat.tile([P, NCHUNK], F32)
            dprod = work.tile([P, NCHUNK, D], BF16, tag="sqd")
            dve(nc.vector.tensor_mul(
                out=dprod, in0=zi_bf, in1=zj_f[0][:, :NCHUNK, :]
            ))
            dve(nc.vector.tensor_reduce(
                out=dt, in_=dprod, axis=AX.X, op=ALU.add
            ))
            dg0 = stat.tile([P, NCHUNK], F32)
            dve(nc.vector.tensor_mul(out=dg0, in0=dt, in1=t_d))
            dg = stat.tile([P, NCHUNK], F32)
            dve(nc.vector.tensor_mul(out=dg, in0=dg0, in1=s2))

            # --- lse = ln(S) via bit-trick + one Newton refinement
            rs0 = stat.tile([P, NCHUNK], F32)
            dve(nc.vector.tensor_add(
                out=rs0, in0=lse_parts[:, :NCHUNK],
                in1=lse_parts[:, NCHUNK : 2 * NCHUNK],
            ))
            rs1 = stat.tile([P, NCHUNK], F32)
            dve(nc.vector.tensor_add(
                out=rs1, in0=lse_parts[:, 2 * NCHUNK : 3 * NCHUNK],
                in1=lse_parts[:, 3 * NCHUNK :],
            ))
            rs = stat.tile([P, NCHUNK], F32)
            dve(nc.vector.tensor_add(out=rs, in0=rs0, in1=rs1))
            vf = stat.tile([P, NCHUNK], F32)
            dve(nc.vector.tensor_copy(out=vf, in_=rs.bitcast(I32)))
            lse0 = stat.tile([P, NCHUNK], F32)
            dve(nc.vector.tensor_scalar(
                out=lse0, in0=vf, scalar1=LN_B, scalar2=LN_S,
                op0=ALU.subtract, op1=ALU.mult,
            ))
            ef = stat.tile([P, NCHUNK], F32)
            nc.scalar.activation(out=ef, in_=lse0, func=AF.Exp, scale=-1.0)
            rf = stat.tile([P, NCHUNK], F32)
            dve(nc.vector.tensor_mul(out=rf, in0=rs, in1=ef))
            uf = stat.tile([P, NCHUNK], F32)
            dve(nc.vector.tensor_scalar(
                out=uf, in0=rf, scalar1=1.0, scalar2=None, op0=ALU.subtract
            ))
            qf = stat.tile([P, NCHUNK], F32)
            dve(nc.vector.scalar_tensor_tensor(
                out=qf, in0=uf, scalar=-0.5, in1=uf,
                op0=ALU.mult, op1=ALU.mult,
            ))
            l1 = stat.tile([P, NCHUNK], F32)
            dve(nc.vector.tensor_add(out=l1, in0=lse0, in1=uf))
            lse = stat.tile([P, NCHUNK], F32)
            dve(nc.vector.tensor_add(out=lse, in0=l1, in1=qf))
            contrib = stat.tile([P, NCHUNK], F32)
            dve(nc.vector.tensor_sub(out=contrib, in0=lse, in1=dg))

            # --- partition reduction via ones-matmul: [1, 4] partials
            pt_fin = psum.tile([P, 1024], F32, tag="pt")
            nc.tensor.matmul(
                pt_fin[:1, :NCHUNK], lhsT=ones, rhs=contrib, start=True, stop=True
            )
            osb = stat.tile([1, NCHUNK], F32)
            nc.vector.tensor_copy(out=osb, in_=pt_fin[:1, :NCHUNK])
            nc.sync.dma_start(out=out, in_=osb)

    nc.compile()
    return nc


_NC = None


def _get_nc():
    global _NC
    if _NC is None:
        _NC = build_nc()
    return _NC


def make_in_maps(z_i: np.ndarray, z_j: np.ndarray):
    z_i = np.asarray(z_i).astype(ml_dtypes.bfloat16)
    z_j = np.asarray(z_j).astype(ml_dtypes.bfloat16)
    in_maps = []
    for c in range(NCORES):
        sl = slice(c * NL, (c + 1) * NL)
        # rotate so each core's own diagonal block lands at rows 0-511;
        # the softmax denominator is invariant to column order.
        zj_rot = np.ascontiguousarray(np.roll(z_j, -c * NL, axis=0))
        in_maps.append(
            {
                "z_i": np.ascontiguousarray(z_i[sl]),
                "z_j": zj_rot,
            }
        )
    return in_maps


def kernel(z_i: np.ndarray, z_j: np.ndarray, **_unused) -> np.ndarray:
    nc = _get_nc()
    in_maps = make_in_maps(z_i, z_j)
    res = bass_utils.run_bass_kernel_spmd(
        nc, in_maps, core_ids=list(range(NCORES))
    )
    total = 0.0
    for c in range(NCORES):
        total += float(res.results[c]["out"].astype(np.float64).sum())
    return np.float32(total / N)
